# revision 6
# baseline (speedup 1.0000x reference)
"""Distributed Trainium2 kernel for AdvancedMultiHeadAttention.

B=2, T=2048, C=1024, H=16 heads, D=64. Causal SDPA with RoPE.
Sharding: data-parallel over batch (cores 0-3 = batch 0, 4-7 = batch 1),
tensor-parallel over heads within each group (4 heads/core). Wo is
row-sharded; partial outputs are summed with a bf16 ReduceScatter per
4-core replica group (2 chunks, overlapped with compute), host concat.
"""

import sys

sys.path.insert(0, "/opt/trn_rl_repo")

import ml_dtypes
import numpy as np

B, T, C = 2, 2048, 1024
H, D = 16, 64
NCORES = 8
HLOC = 4            # heads per core
CLOC = HLOC * D     # 256 attention dims per core
NSTRIP = C // 128   # 8 contraction strips
NSPAN = T // 512    # 4 query spans
NT = T // 128       # 16 row tiles
RG = [[0, 1, 2, 3], [4, 5, 6, 7]]

_CACHE = {}


def _build_nc(rs=True, reps=1):
    import concourse.bacc as bacc
    import concourse.mybir as mybir
    import concourse.tile as tile

    f32 = mybir.dt.float32
    bf16 = mybir.dt.bfloat16
    AF = mybir.ActivationFunctionType

    nc = bacc.Bacc("TRN2", target_bir_lowering=False, debug=False,
                   num_devices=NCORES)

    xbT = nc.declare_dram_parameter("xbT", [C, T], bf16, isOutput=False)
    wqkvT = nc.declare_dram_parameter("wqkvT", [C, 3 * CLOC], bf16, isOutput=False)
    woT = nc.declare_dram_parameter("woT", [CLOC, C], bf16, isOutput=False)
    csT = nc.declare_dram_parameter("csT", [128, 2 * T], bf16, isOutput=False)
    band = nc.declare_dram_parameter("band", [128, 896], bf16, isOutput=False)
    outx = nc.declare_dram_parameter("out", [512, C], bf16, isOutput=True)

    with tile.TileContext(nc) as tc:
        with (
            tc.tile_pool(name="cst", bufs=1) as cst,
            tc.tile_pool(name="work", bufs=1) as work,
            tc.tile_pool(name="ps", bufs=1, space="PSUM") as ps,
            tc.tile_pool(name="dram", bufs=1, space="DRAM") as dram,
        ):
            # ---- loads (order matches first use; coalesced DMAs) ----
            wqkv_sb = [cst.tile([128, 3 * CLOC], bf16, tag=f"wqkv{i}",
                                name=f"wqkv{i}") for i in range(NSTRIP)]
            xt = [[cst.tile([128, 1024], bf16, tag=f"xt{i}_{s}", name=f"xt{i}_{s}")
                   for s in range(2)] for i in range(NSTRIP)]
            for i in range(NSTRIP):
                nc.sync.dma_start(wqkv_sb[i][:], wqkvT[i * 128:(i + 1) * 128, :])
            for i in range(NSTRIP):
                nc.sync.dma_start(xt[i][0][:], xbT[i * 128:(i + 1) * 128, 0:1024])
            cs_sb = cst.tile([128, 2 * T], bf16, tag="cs", name="cs")
            nc.sync.dma_start(cs_sb[:], csT[:, :])
            for i in range(NSTRIP):
                nc.sync.dma_start(xt[i][1][:],
                                  xbT[i * 128:(i + 1) * 128, 1024:2048])
            band_sb = cst.tile([128, 896], bf16, tag="band", name="band")
            nc.sync.dma_start(band_sb[:], band[:, :])
            wo_sb = [cst.tile([128, C], bf16, tag=f"wo{i}", name=f"wo{i}") for i in range(2)]
            for i in range(2):
                nc.sync.dma_start(wo_sb[i][:], woT[i * 128:(i + 1) * 128, :])
            wq_sb = [w[:, 0:CLOC] for w in wqkv_sb]
            wk_sb = [w[:, CLOC:2 * CLOC] for w in wqkv_sb]
            wv_sb = [w[:, 2 * CLOC:3 * CLOC] for w in wqkv_sb]
            cos_sb = cs_sb[:, 0:T]
            sin_sb = cs_sb[:, T:2 * T]

            ONES = work.tile([1, 64], bf16, tag="ones", name="ones")
            nc.vector.memset(ONES[:], 1.0)
            TOPq = work.tile([128, T], bf16, tag="topq", name="topq")
            BOTq = work.tile([128, T], bf16, tag="botq", name="botq")
            TOPk = work.tile([128, T], bf16, tag="topk", name="topk")
            BOTk = work.tile([128, T], bf16, tag="botk", name="botk")
            # per-head-contiguous [top;bot] pair tiles for K=64 S matmuls
            QP = [work.tile([128, T], bf16, tag=f"qp{p}", name=f"qp{p}")
                  for p in range(2)]
            KP = [work.tile([128, T], bf16, tag=f"kp{p}", name=f"kp{p}")
                  for p in range(2)]

            for _rep in range(reps):
                OT = [work.tile([128, T], bf16, tag=f"ot{i}", name=f"ot{i}") for i in range(2)]

                def qk_proj(sp, wsb, TOPt, BOTt, tgE, tgO, pbufs):
                    qs = slice(sp * 512, (sp + 1) * 512)
                    psE = ps.tile([128, 512], f32, tag=tgE, name="psE", bufs=pbufs)
                    psO = ps.tile([128, 512], f32, tag=tgO, name="psO", bufs=pbufs)
                    xsl = [xt[ci][sp // 2][:, (sp % 2) * 512:(sp % 2 + 1) * 512]
                           for ci in range(NSTRIP)]
                    for ci in range(NSTRIP):
                        nc.tensor.matmul(psE[:], wsb[ci][:, 0:128], xsl[ci],
                                         start=(ci == 0), stop=(ci == NSTRIP - 1))
                    for ci in range(NSTRIP):
                        nc.tensor.matmul(psO[:], wsb[ci][:, 128:256], xsl[ci],
                                         start=(ci == 0), stop=(ci == NSTRIP - 1))
                    t1 = work.tile([128, 512], bf16, tag="t1", name="t1", bufs=3)
                    t2 = work.tile([128, 512], bf16, tag="t2", name="t2", bufs=3)
                    t3 = work.tile([128, 512], bf16, tag="t3", name="t3", bufs=3)
                    t4 = work.tile([128, 512], bf16, tag="t4", name="t4", bufs=3)
                    with nc.allow_low_precision(reason="rope in bf16"):
                        nc.vector.tensor_mul(t1[:], psE[:], cos_sb[:, qs])
                        nc.vector.tensor_mul(t2[:], psO[:], sin_sb[:, qs])
                        nc.vector.tensor_sub(TOPt[:, qs], t1[:], t2[:])
                        nc.vector.tensor_mul(t3[:], psE[:], sin_sb[:, qs])
                        nc.vector.tensor_mul(t4[:], psO[:], cos_sb[:, qs])
                        nc.vector.tensor_add(BOTt[:, qs], t3[:], t4[:])
                    PAIR = QP if TOPt is TOPq else KP
                    for h in range(HLOC):
                        p, hh = h // 2, h % 2
                        nc.gpsimd.tensor_copy(
                            PAIR[p][hh * 64:hh * 64 + 32, qs],
                            TOPt[h * 32:(h + 1) * 32, qs])
                        if sp < 2:
                            nc.scalar.copy(
                                PAIR[p][hh * 64 + 32:hh * 64 + 64, qs],
                                BOTt[h * 32:(h + 1) * 32, qs])
                        else:
                            nc.gpsimd.tensor_copy(
                                PAIR[p][hh * 64 + 32:hh * 64 + 64, qs],
                                BOTt[h * 32:(h + 1) * 32, qs])

                def wo_chunk(c):
                    yb = dram.tile([1024, C], bf16, tag=f"yb{c}_{_rep}", name=f"yb{c}_{_rep}")
                    for ttl in range(8):
                        tt = c * 8 + ttl
                        ysb = work.tile([128, C], bf16, tag="ysb", name="ysb", bufs=4)
                        for cs in range(2):
                            psY = ps.tile([128, 512], f32, name="psY",
                                          tag=("s0" if cs == 0 else "s1"), bufs=2)
                            for s2 in range(2):
                                nc.tensor.matmul(
                                    psY[:], OT[s2][:, tt * 128:(tt + 1) * 128],
                                    wo_sb[s2][:, cs * 512:(cs + 1) * 512],
                                    start=(s2 == 0), stop=(s2 == 1))
                            if cs == 0:
                                nc.vector.tensor_copy(
                                    ysb[:, cs * 512:(cs + 1) * 512], psY[:])
                            else:
                                nc.scalar.copy(
                                    ysb[:, cs * 512:(cs + 1) * 512], psY[:])
                        nc.sync.dma_start(yb[ttl * 128:(ttl + 1) * 128, :], ysb[:])
                    if rs:
                        rst = dram.tile([256, C], bf16, tag=f"rs{c}_{_rep}", name=f"rs{c}_{_rep}")
                        nc.gpsimd.collective_compute(
                            "ReduceScatter", mybir.AluOpType.add, replica_groups=RG,
                            ins=[yb[:].opt()], outs=[rst[:].opt()])
                        nc.sync.dma_start(outx[c * 256:(c + 1) * 256, :], rst[:])
                    else:
                        nc.sync.dma_start(outx[c * 256:(c + 1) * 256, :],
                                          yb[c * 256:(c + 1) * 256, :])

                def attn_subpass(sp, sub):
                    qs = slice(sp * 512, (sp + 1) * 512)
                    nkt = (sp + 1) * 4
                    psO2 = [ps.tile([65, 512], f32, tag=f"o{2 * sub + i}",
                                    name=f"psAcc{2 * sub + i}") for i in (0, 1)]
                    prev = None

                    def flush(prev):
                        ats, pkt = prev
                        for i in (0, 1):
                            h = 2 * sub + i
                            nc.tensor.matmul(psO2[i][:],
                                             vaug[pkt][:, h * 65:(h + 1) * 65],
                                             ats[i][:],
                                             start=(pkt == 0),
                                             stop=(pkt == nkt - 1))

                    for kt in range(nkt):
                        psS = [ps.tile([128, 512], f32, tag=f"s{i}",
                                       name=f"s{i}", bufs=2) for i in (0, 1)]
                        for i in (0, 1):
                            nc.tensor.matmul(
                                psS[i][:],
                                KP[sub][i * 64:(i + 1) * 64,
                                        kt * 128:(kt + 1) * 128],
                                QP[sub][i * 64:(i + 1) * 64, qs],
                                start=True, stop=True,
                                tile_position=(64 * i, 0))
                        ats = []
                        delta = kt * 128 - sp * 512
                        for i in (0, 1):
                            at = work.tile([128, 512], bf16, tag=f"at{sub}{i}",
                                           name=f"at{sub}{i}", bufs=3)
                            if delta >= 0:
                                if delta > 0:
                                    nc.vector.memset(at[:, 0:delta], 0.0)
                                nc.scalar.activation(at[:, delta:512],
                                                     psS[i][:, delta:512],
                                                     AF.Exp, scale=0.125)
                                nc.vector.tensor_mul(
                                    at[:, delta:delta + 128],
                                    at[:, delta:delta + 128],
                                    band_sb[:, 384:512])
                            else:
                                nc.scalar.activation(at[:], psS[i][:], AF.Exp,
                                                     scale=0.125)
                            ats.append(at)
                        if prev is not None:
                            flush(prev)
                        prev = (ats, kt)
                    flush(prev)

                    # normalize: r = 1/rowsum broadcast via bf16 K=1 outer MM
                    for i in (0, 1):
                        h = 2 * sub + i
                        r1 = work.tile([1, 512], bf16, tag=f"rer{h}",
                                       name=f"rer{h}", bufs=3)
                        with nc.allow_low_precision(reason="softmax scale bf16"):
                            nc.vector.reciprocal(r1[:], psO2[i][64:65, :])
                        psR = ps.tile([64, 512], f32, tag=f"s{i}", name="psR",
                                      bufs=2)
                        nc.tensor.matmul(psR[:], ONES[:], r1[:],
                                         start=True, stop=True)
                        rb = work.tile([64, 512], bf16, tag=f"rb{i}",
                                       name=f"rb{i}", bufs=3)
                        nc.vector.tensor_copy(rb[:], psR[:])
                        nc.vector.tensor_mul(OT[sub][i * 64:(i + 1) * 64, qs],
                                             psO2[i][0:64, :], rb[:])

                vaug = [None] * NT

                def v_proj(tts):
                    for tt in tts:
                        psV = ps.tile([128, CLOC], f32,
                                      tag=("o2" if tt % 2 == 0 else "o3"),
                                      name="psV")
                        for ci in range(NSTRIP):
                            nc.tensor.matmul(
                                psV[:],
                                xt[ci][tt // 8][:, (tt % 8) * 128:(tt % 8 + 1) * 128],
                                wv_sb[ci],
                                start=(ci == 0), stop=(ci == NSTRIP - 1))
                        va = work.tile([128, HLOC * 65], bf16, tag=f"va{tt}",
                                       name=f"va{tt}")
                        va_v = va[:].rearrange("p (h d) -> p h d", h=HLOC)
                        nc.vector.tensor_copy(
                            va_v[:, :, 0:64],
                            psV[:].rearrange("p (h d) -> p h d", h=HLOC))
                        nc.vector.memset(va_v[:, :, 64:65], 1.0)
                        vaug[tt] = va

                # software-pipelined spans: proj runs 2 spans ahead of attn
                def proj(sp):
                    qk_proj(sp, wq_sb, TOPq, BOTq, "s0", "s1", 2)
                    qk_proj(sp, wk_sb, TOPk, BOTk, "s0", "s1", 2)

                proj(0)
                proj(1)
                v_proj(range(0, 8))
                attn_subpass(0, 0)
                attn_subpass(0, 1)
                proj(2)
                v_proj(range(8, 12))
                attn_subpass(1, 0)
                attn_subpass(1, 1)
                wo_chunk(0)
                proj(3)
                v_proj(range(12, 16))
                attn_subpass(2, 0)
                attn_subpass(2, 1)
                attn_subpass(3, 0)
                attn_subpass(3, 1)
                wo_chunk(1)

    nc.compile()
    return nc


def _host_tables():
    bf = ml_dtypes.bfloat16
    j = np.arange(0, D, 2, dtype=np.float64)
    inv = 1.0 / (10000.0 ** (j / D))
    t = np.arange(T, dtype=np.float64)
    fr = np.outer(t, inv)                      # [T, 32]
    cosT = np.tile(np.cos(fr).T, (4, 1))       # [128, T]
    sinT = np.tile(np.sin(fr).T, (4, 1))
    csT = np.concatenate([cosT, sinT], axis=1).astype(bf)  # [128, 2T]
    k = np.arange(128)[:, None]
    c = np.arange(896)[None, :]
    band = (c >= k + 384).astype(bf)
    return csT, band


def _in_maps(x, Wq, Wk, Wv, Wo):
    bf = ml_dtypes.bfloat16
    csT, band = _host_tables()
    maps = []
    for core in range(NCORES):
        b = core // 4
        g0 = HLOC * (core % 4)
        heads = range(g0, g0 + HLOC)
        evens = np.concatenate([g * 64 + np.arange(0, 64, 2) for g in heads])
        odds = np.concatenate([g * 64 + np.arange(1, 64, 2) for g in heads])
        perm = np.concatenate([evens, odds])
        vrows = np.concatenate([np.arange(g * 64, (g + 1) * 64) for g in heads])
        wqkv = np.concatenate(
            [Wq[perm].T, Wk[perm].T, Wv[vrows].T], axis=1)  # [C, 3*CLOC]
        maps.append({
            "xbT": np.ascontiguousarray(x[b].T).astype(bf),
            "wqkvT": np.ascontiguousarray(wqkv).astype(bf),
            "woT": np.ascontiguousarray(Wo[:, vrows].T).astype(bf),
            "csT": csT, "band": band,
        })
    return maps


def _run(x, Wq, Wk, Wv, Wo, trace=False):
    from concourse.bass_utils import run_bass_kernel_spmd

    if "nc" not in _CACHE:
        _CACHE["nc"] = _build_nc()
    nc = _CACHE["nc"]
    maps = _in_maps(x, Wq, Wk, Wv, Wo)
    return run_bass_kernel_spmd(nc, maps, list(range(NCORES)), trace=trace)


def kernel(x, Wq, Wk, Wv, Wo):
    x = np.asarray(x, dtype=np.float32)
    res = _run(x, np.asarray(Wq, np.float32), np.asarray(Wk, np.float32),
               np.asarray(Wv, np.float32), np.asarray(Wo, np.float32))
    y = np.zeros((B, T, C), np.float32)
    for core in range(NCORES):
        b, r = core // 4, core % 4
        o = np.asarray(res.results[core]["out"]).astype(np.float32)
        y[b, r * 256:(r + 1) * 256] = o[0:256]
        y[b, 1024 + r * 256:1024 + (r + 1) * 256] = o[256:512]
    return y



# revision 14
# speedup vs baseline: 1.0208x; 1.0208x over previous
"""Distributed Trainium2 kernel for AdvancedMultiHeadAttention.

B=2, T=2048, C=1024, H=16 heads, D=64. Causal SDPA with RoPE.
Sharding: data-parallel over batch (cores 0-3 = batch 0, 4-7 = batch 1),
tensor-parallel over heads within each group (4 heads/core). Wo is
row-sharded; partial outputs are summed with a bf16 ReduceScatter per
4-core replica group (2 chunks, overlapped with compute), host concat.
"""

import sys

sys.path.insert(0, "/opt/trn_rl_repo")

import ml_dtypes
import numpy as np

B, T, C = 2, 2048, 1024
H, D = 16, 64
NCORES = 8
HLOC = 4            # heads per core
CLOC = HLOC * D     # 256 attention dims per core
NSTRIP = C // 128   # 8 contraction strips
NSPAN = T // 512    # 4 query spans
NT = T // 128       # 16 row tiles
RG = [[0, 1, 2, 3], [4, 5, 6, 7]]

_CACHE = {}


def _build_nc(rs=True, reps=1):
    import concourse.bacc as bacc
    import concourse.mybir as mybir
    import concourse.tile as tile

    f32 = mybir.dt.float32
    bf16 = mybir.dt.bfloat16
    AF = mybir.ActivationFunctionType

    nc = bacc.Bacc("TRN2", target_bir_lowering=False, debug=False,
                   num_devices=NCORES)

    xbT = nc.declare_dram_parameter("xbT", [C, T], bf16, isOutput=False)
    wqkvT = nc.declare_dram_parameter("wqkvT", [C, 3 * CLOC], bf16, isOutput=False)
    woT = nc.declare_dram_parameter("woT", [CLOC, C], bf16, isOutput=False)
    csT = nc.declare_dram_parameter("csT", [128, 2 * T], bf16, isOutput=False)
    band = nc.declare_dram_parameter("band", [128, 896], bf16, isOutput=False)
    outx = nc.declare_dram_parameter("out", [512, C], bf16, isOutput=True)

    with tile.TileContext(nc) as tc:
        with (
            tc.tile_pool(name="cst", bufs=1) as cst,
            tc.tile_pool(name="work", bufs=1) as work,
            tc.tile_pool(name="ps", bufs=1, space="PSUM") as ps,
            tc.tile_pool(name="dram", bufs=1, space="DRAM") as dram,
        ):
            # ---- loads (order matches first use; coalesced DMAs) ----
            wqkv_sb = [cst.tile([128, 3 * CLOC], bf16, tag=f"wqkv{i}",
                                name=f"wqkv{i}") for i in range(NSTRIP)]
            xt = [[cst.tile([128, 1024], bf16, tag=f"xt{i}_{s}", name=f"xt{i}_{s}")
                   for s in range(2)] for i in range(NSTRIP)]
            for i in range(NSTRIP):
                nc.sync.dma_start(wqkv_sb[i][:], wqkvT[i * 128:(i + 1) * 128, :])
            for i in range(NSTRIP):
                nc.sync.dma_start(xt[i][0][:], xbT[i * 128:(i + 1) * 128, 0:1024])
            cs_sb = cst.tile([128, 2 * T], bf16, tag="cs", name="cs")
            nc.sync.dma_start(cs_sb[:], csT[:, :])
            for i in range(NSTRIP):
                nc.sync.dma_start(xt[i][1][:],
                                  xbT[i * 128:(i + 1) * 128, 1024:2048])
            band_sb = cst.tile([128, 896], bf16, tag="band", name="band")
            nc.sync.dma_start(band_sb[:], band[:, :])
            wo_sb = [cst.tile([128, C], bf16, tag=f"wo{i}", name=f"wo{i}") for i in range(2)]
            for i in range(2):
                nc.sync.dma_start(wo_sb[i][:], woT[i * 128:(i + 1) * 128, :])
            wq_sb = [w[:, 0:CLOC] for w in wqkv_sb]
            wk_sb = [w[:, CLOC:2 * CLOC] for w in wqkv_sb]
            wv_sb = [w[:, 2 * CLOC:3 * CLOC] for w in wqkv_sb]
            cos_sb = cs_sb[:, 0:T]
            sin_sb = cs_sb[:, T:2 * T]

            ONES = work.tile([1, 64], bf16, tag="ones", name="ones")
            nc.vector.memset(ONES[:], 1.0)
            TOPq = work.tile([128, T], bf16, tag="topq", name="topq")
            BOTq = work.tile([128, T], bf16, tag="botq", name="botq")
            TOPk = work.tile([128, T], bf16, tag="topk", name="topk")
            BOTk = work.tile([128, T], bf16, tag="botk", name="botk")
            # per-head-contiguous [top;bot] pair tiles for K=64 S matmuls
            QP = [work.tile([128, T], bf16, tag=f"qp{p}", name=f"qp{p}")
                  for p in range(2)]
            KP = [work.tile([128, T], bf16, tag=f"kp{p}", name=f"kp{p}")
                  for p in range(2)]

            for _rep in range(reps):
                OT = [work.tile([128, T], bf16, tag=f"ot{i}", name=f"ot{i}") for i in range(2)]

                def qk_proj(sp, wsb, TOPt, BOTt, tgE, tgO, pbufs):
                    qs = slice(sp * 512, (sp + 1) * 512)
                    psE = ps.tile([128, 512], f32, tag=tgE, name="psE", bufs=pbufs)
                    psO = ps.tile([128, 512], f32, tag=tgO, name="psO", bufs=pbufs)
                    xsl = [xt[ci][sp // 2][:, (sp % 2) * 512:(sp % 2 + 1) * 512]
                           for ci in range(NSTRIP)]
                    for ci in range(NSTRIP):
                        nc.tensor.matmul(psE[:], wsb[ci][:, 0:128], xsl[ci],
                                         start=(ci == 0), stop=(ci == NSTRIP - 1))
                    for ci in range(NSTRIP):
                        nc.tensor.matmul(psO[:], wsb[ci][:, 128:256], xsl[ci],
                                         start=(ci == 0), stop=(ci == NSTRIP - 1))
                    t1 = work.tile([128, 512], bf16, tag="t1", name="t1", bufs=3)
                    t2 = work.tile([128, 512], bf16, tag="t2", name="t2", bufs=3)
                    t3 = work.tile([128, 512], bf16, tag="t3", name="t3", bufs=3)
                    t4 = work.tile([128, 512], bf16, tag="t4", name="t4", bufs=3)
                    with nc.allow_low_precision(reason="rope in bf16"):
                        nc.vector.tensor_mul(t1[:], psE[:], cos_sb[:, qs])
                        nc.vector.tensor_mul(t2[:], psO[:], sin_sb[:, qs])
                        nc.vector.tensor_sub(TOPt[:, qs], t1[:], t2[:])
                        nc.vector.tensor_mul(t3[:], psE[:], sin_sb[:, qs])
                        nc.vector.tensor_mul(t4[:], psO[:], cos_sb[:, qs])
                        nc.vector.tensor_add(BOTt[:, qs], t3[:], t4[:])
                    PAIR = QP if TOPt is TOPq else KP
                    for h in range(HLOC):
                        p, hh = h // 2, h % 2
                        nc.gpsimd.tensor_copy(
                            PAIR[p][hh * 64:hh * 64 + 32, qs],
                            TOPt[h * 32:(h + 1) * 32, qs])
                        if sp < 2:
                            nc.scalar.copy(
                                PAIR[p][hh * 64 + 32:hh * 64 + 64, qs],
                                BOTt[h * 32:(h + 1) * 32, qs])
                        else:
                            nc.gpsimd.tensor_copy(
                                PAIR[p][hh * 64 + 32:hh * 64 + 64, qs],
                                BOTt[h * 32:(h + 1) * 32, qs])

                def wo_chunk(c):
                    yb = dram.tile([1024, C], bf16, tag=f"yb{c}_{_rep}", name=f"yb{c}_{_rep}")
                    for ttl in range(8):
                        tt = c * 8 + ttl
                        ysb = work.tile([128, C], bf16, tag="ysb", name="ysb", bufs=4)
                        for cs in range(2):
                            psY = ps.tile([128, 512], f32, name="psY",
                                          tag=("s0" if cs == 0 else "s1"), bufs=2)
                            for s2 in range(2):
                                nc.tensor.matmul(
                                    psY[:], OT[s2][:, tt * 128:(tt + 1) * 128],
                                    wo_sb[s2][:, cs * 512:(cs + 1) * 512],
                                    start=(s2 == 0), stop=(s2 == 1))
                            if cs == 0:
                                nc.vector.tensor_copy(
                                    ysb[:, cs * 512:(cs + 1) * 512], psY[:])
                            else:
                                nc.scalar.copy(
                                    ysb[:, cs * 512:(cs + 1) * 512], psY[:])
                        nc.sync.dma_start(yb[ttl * 128:(ttl + 1) * 128, :], ysb[:])
                    if rs:
                        rst = dram.tile([256, C], bf16, tag=f"rs{c}_{_rep}", name=f"rs{c}_{_rep}")
                        nc.gpsimd.collective_compute(
                            "ReduceScatter", mybir.AluOpType.add, replica_groups=RG,
                            ins=[yb[:].opt()], outs=[rst[:].opt()])
                        nc.sync.dma_start(outx[c * 256:(c + 1) * 256, :], rst[:])
                    else:
                        nc.sync.dma_start(outx[c * 256:(c + 1) * 256, :],
                                          yb[c * 256:(c + 1) * 256, :])

                def attn_subpass(sp, sub):
                    qs = slice(sp * 512, (sp + 1) * 512)
                    nkt = (sp + 1) * 4
                    psO2 = [ps.tile([65, 512], f32, tag=f"o{2 * sub + i}",
                                    name=f"psAcc{2 * sub + i}") for i in (0, 1)]
                    prev = None

                    def flush(prev):
                        ats, pkt = prev
                        for i in (0, 1):
                            h = 2 * sub + i
                            nc.tensor.matmul(psO2[i][:],
                                             vaug[pkt][:, h * 65:(h + 1) * 65],
                                             ats[i][:],
                                             start=(pkt == 0),
                                             stop=(pkt == nkt - 1))

                    for kt in range(nkt):
                        psS = [ps.tile([128, 512], f32, tag=f"s{i}",
                                       name=f"s{i}", bufs=2) for i in (0, 1)]
                        for i in (0, 1):
                            nc.tensor.matmul(
                                psS[i][:],
                                KP[sub][i * 64:(i + 1) * 64,
                                        kt * 128:(kt + 1) * 128],
                                QP[sub][i * 64:(i + 1) * 64, qs],
                                start=True, stop=True,
                                tile_position=(64 * i, 0))
                        ats = []
                        delta = kt * 128 - sp * 512
                        for i in (0, 1):
                            at = work.tile([128, 512], bf16, tag=f"at{sub}{i}",
                                           name=f"at{sub}{i}", bufs=3)
                            if delta >= 0:
                                if delta > 0:
                                    nc.vector.memset(at[:, 0:delta], 0.0)
                                nc.scalar.activation(at[:, delta:512],
                                                     psS[i][:, delta:512],
                                                     AF.Exp, scale=0.125)
                                nc.vector.tensor_mul(
                                    at[:, delta:delta + 128],
                                    at[:, delta:delta + 128],
                                    band_sb[:, 384:512])
                            else:
                                nc.scalar.activation(at[:], psS[i][:], AF.Exp,
                                                     scale=0.125)
                            ats.append(at)
                        if prev is not None:
                            flush(prev)
                        prev = (ats, kt)
                    flush(prev)

                    # normalize: r = 1/rowsum broadcast via bf16 K=1 outer MM
                    for i in (0, 1):
                        h = 2 * sub + i
                        r1 = work.tile([1, 512], bf16, tag=f"rer{h}",
                                       name=f"rer{h}", bufs=3)
                        with nc.allow_low_precision(reason="softmax scale bf16"):
                            nc.vector.reciprocal(r1[:], psO2[i][64:65, :])
                        psR = ps.tile([64, 512], f32, tag=f"s{i}", name="psR",
                                      bufs=2)
                        nc.tensor.matmul(psR[:], ONES[:], r1[:],
                                         start=True, stop=True)
                        rb = work.tile([64, 512], bf16, tag=f"rb{i}",
                                       name=f"rb{i}", bufs=3)
                        nc.vector.tensor_copy(rb[:], psR[:])
                        nc.vector.tensor_mul(OT[sub][i * 64:(i + 1) * 64, qs],
                                             psO2[i][0:64, :], rb[:])

                vaug = [None] * NT

                def v_proj(tts):
                    for tt in tts:
                        psV = ps.tile([128, CLOC], f32,
                                      tag=("o2" if tt % 2 == 0 else "o3"),
                                      name="psV")
                        for ci in range(NSTRIP):
                            nc.tensor.matmul(
                                psV[:],
                                xt[ci][tt // 8][:, (tt % 8) * 128:(tt % 8 + 1) * 128],
                                wv_sb[ci],
                                start=(ci == 0), stop=(ci == NSTRIP - 1))
                        va = work.tile([128, HLOC * 65], bf16, tag=f"va{tt}",
                                       name=f"va{tt}")
                        va_v = va[:].rearrange("p (h d) -> p h d", h=HLOC)
                        nc.vector.tensor_copy(
                            va_v[:, :, 0:64],
                            psV[:].rearrange("p (h d) -> p h d", h=HLOC))
                        nc.vector.memset(va_v[:, :, 64:65], 1.0)
                        vaug[tt] = va

                # software-pipelined spans: proj runs 2 spans ahead of attn
                def proj(sp):
                    qk_proj(sp, wq_sb, TOPq, BOTq, "s0", "s1", 2)
                    qk_proj(sp, wk_sb, TOPk, BOTk, "s0", "s1", 2)

                proj(0)
                proj(1)
                v_proj(range(0, 4))
                attn_subpass(0, 0)
                v_proj(range(4, 8))
                attn_subpass(0, 1)
                proj(2)
                attn_subpass(1, 0)
                v_proj(range(8, 12))
                attn_subpass(1, 1)
                wo_chunk(0)
                proj(3)
                attn_subpass(2, 0)
                v_proj(range(12, 16))
                attn_subpass(2, 1)
                attn_subpass(3, 0)
                attn_subpass(3, 1)
                wo_chunk(1)

    nc.compile()
    return nc


def _host_tables():
    bf = ml_dtypes.bfloat16
    j = np.arange(0, D, 2, dtype=np.float64)
    inv = 1.0 / (10000.0 ** (j / D))
    t = np.arange(T, dtype=np.float64)
    fr = np.outer(t, inv)                      # [T, 32]
    cosT = np.tile(np.cos(fr).T, (4, 1))       # [128, T]
    sinT = np.tile(np.sin(fr).T, (4, 1))
    csT = np.concatenate([cosT, sinT], axis=1).astype(bf)  # [128, 2T]
    k = np.arange(128)[:, None]
    c = np.arange(896)[None, :]
    band = (c >= k + 384).astype(bf)
    return csT, band


def _in_maps(x, Wq, Wk, Wv, Wo):
    bf = ml_dtypes.bfloat16
    csT, band = _host_tables()
    maps = []
    for core in range(NCORES):
        b = core // 4
        g0 = HLOC * (core % 4)
        heads = range(g0, g0 + HLOC)
        evens = np.concatenate([g * 64 + np.arange(0, 64, 2) for g in heads])
        odds = np.concatenate([g * 64 + np.arange(1, 64, 2) for g in heads])
        perm = np.concatenate([evens, odds])
        vrows = np.concatenate([np.arange(g * 64, (g + 1) * 64) for g in heads])
        wqkv = np.concatenate(
            [Wq[perm].T, Wk[perm].T, Wv[vrows].T], axis=1)  # [C, 3*CLOC]
        maps.append({
            "xbT": np.ascontiguousarray(x[b].T).astype(bf),
            "wqkvT": np.ascontiguousarray(wqkv).astype(bf),
            "woT": np.ascontiguousarray(Wo[:, vrows].T).astype(bf),
            "csT": csT, "band": band,
        })
    return maps


def _run(x, Wq, Wk, Wv, Wo, trace=False):
    from concourse.bass_utils import run_bass_kernel_spmd

    if "nc" not in _CACHE:
        _CACHE["nc"] = _build_nc()
    nc = _CACHE["nc"]
    maps = _in_maps(x, Wq, Wk, Wv, Wo)
    return run_bass_kernel_spmd(nc, maps, list(range(NCORES)), trace=trace)


def kernel(x, Wq, Wk, Wv, Wo):
    x = np.asarray(x, dtype=np.float32)
    res = _run(x, np.asarray(Wq, np.float32), np.asarray(Wk, np.float32),
               np.asarray(Wv, np.float32), np.asarray(Wo, np.float32))
    y = np.zeros((B, T, C), np.float32)
    for core in range(NCORES):
        b, r = core // 4, core % 4
        o = np.asarray(res.results[core]["out"]).astype(np.float32)
        y[b, r * 256:(r + 1) * 256] = o[0:256]
        y[b, 1024 + r * 256:1024 + (r + 1) * 256] = o[256:512]
    return y

